# revision 1
# baseline (speedup 1.0000x reference)
import numpy as np
import jax
import jax.numpy as jnp
from functools import partial

NUM_EMB = 100000
EMB = 64
HEADS = 4
ATT = 32
HD = HEADS * ATT  # 128
B = 8192
NCORES = 8
BL = B // NCORES  # 1024 samples per core


def _attn_block(y, QW, Qb, KW, Kb, VW, Vb, RW, Rb):
    b, f, _ = y.shape
    Q = (y @ QW.T + Qb).reshape(b, f, HEADS, ATT)
    K = (y @ KW.T + Kb).reshape(b, f, HEADS, ATT)
    V = (y @ VW.T + Vb).reshape(b, f, HEADS, ATT)
    Res = y @ RW.T + Rb
    scores = jnp.einsum('bqhd,bkhd->bhqk', Q, K)
    A = jax.nn.softmax(scores, axis=-1)
    O = jnp.einsum('bhqk,bkhd->bqhd', A, V).reshape(b, f, HD)
    return jax.nn.relu(O + Res)


@partial(jax.pmap, axis_name='x',
         in_axes=(0, 0, 0, 0, 0) + (None,) * 20)
def _fwd(onehot_i, onehot_x, mh_i, mh_x, ctns,
         xx, xy,
         QW1, Qb1, KW1, Kb1, VW1, Vb1, RW1, Rb1,
         QW2, Qb2, KW2, Kb2, VW2, Vb2, RW2, Rb2,
         logitW, logitb):
    onehot_fields = xx[onehot_i] * onehot_x[..., None]          # [BL,20,EMB]
    mh_fields = (xx[mh_i] * mh_x[..., None]).sum(axis=2)        # [2,BL,EMB]
    mh_fields = jnp.transpose(mh_fields, (1, 0, 2))             # [BL,2,EMB]
    ctns_fields = ctns[..., None] * xy                          # [BL,10,EMB]
    y = jnp.concatenate([onehot_fields, mh_fields, ctns_fields], axis=1)
    y = _attn_block(y, QW1, Qb1, KW1, Kb1, VW1, Vb1, RW1, Rb1)
    y = _attn_block(y, QW2, Qb2, KW2, Kb2, VW2, Vb2, RW2, Rb2)
    flat = y.reshape(y.shape[0], -1)
    out = jax.nn.sigmoid(flat @ logitW.T + logitb)
    return out.squeeze(-1)


def kernel(**inputs) -> np.ndarray:
    f32 = lambda k: np.asarray(inputs[k], np.float32)
    i32 = lambda k: np.asarray(inputs[k], np.int32)

    onehot_i = i32('onehot_i').reshape(NCORES, BL, 20)
    onehot_x = f32('onehot_x').reshape(NCORES, BL, 20)
    # mh_i/mh_x are [2, B, 50] -> shard over batch dim, keep leading 2
    mh_i = np.transpose(i32('mh_i').reshape(2, NCORES, BL, 50), (1, 0, 2, 3))
    mh_x = np.transpose(f32('mh_x').reshape(2, NCORES, BL, 50), (1, 0, 2, 3))
    ctns = f32('ctns').reshape(NCORES, BL, -1)

    rep = [f32(k) for k in (
        'xx', 'xy',
        'QW1', 'Qb1', 'KW1', 'Kb1', 'VW1', 'Vb1', 'RW1', 'Rb1',
        'QW2', 'Qb2', 'KW2', 'Kb2', 'VW2', 'Vb2', 'RW2', 'Rb2',
        'logitW', 'logitb')]

    out = _fwd(onehot_i, onehot_x, mh_i, mh_x, ctns, *rep)
    return np.asarray(out, np.float32).reshape(B)

